# revision 1
# baseline (speedup 1.0000x reference)
"""Trainium2 Bass kernel for temporal-window GNN mean aggregation.

    out = x + scatter_mean(x[src] * mask, dst),
    mask = (edge_time <= seed_time[dst]) & (edge_time > seed_time[dst] - 100)

Sharding: destination-node sharding across 8 cores (no collectives).
Host work is layout only: sort edges by (dst window, src bank), pad to a
uniform slot grid, build int16 gather-index planes (mask-independent), and
ship per-slot metadata (edge_time, seed_time[dst], dst%128).  All reference
arithmetic - the temporal mask compare, the masked segment sums / counts
(one-hot matmul on the PE array), the divide and the residual add - happens
on device.

Device per core (SPMD, one program):
  phase 0: wide DVE ops compute mask m per slot and fold it into the
           one-hot key dl_eff = (dst%128)+300-300*m (no iota match -> S=0).
  loop over chunks of CW windows (window = 128 consecutive dst):
    - 4x dma_gather on 4 SWDGE queues (one per 25089-row src bank, int16
      index limit) fetch 512-byte x16 rows = [128 features, 1.0, pad] for
      every slot; the ones column feeds the count accumulation
    - one batched DVE tensor_tensor builds the one-hot S = (iota == dl_eff)
    - PE per window: K matmuls accumulate PSUM[dst, 0:129] += S^T @ G
    - counts: max(cnt,1) + reciprocal batched per chunk, mean via ACT scale
    - residual: out rows += x rows via one accumulate-DMA per chunk
"""

import math
import sys

import numpy as np

for _p in ("/opt/trn_rl_repo",):
    if _p not in sys.path:
        sys.path.insert(0, _p)

import concourse.bass as bass
import concourse.mybir as mybir
import concourse.tile as tile
from concourse import bacc
from concourse.bass_utils import run_bass_kernel_spmd

P = 128            # SBUF partitions == dst-window size == edge-block size
D = 128            # feature dim
NCORES = 8
W = 98             # dst windows per core
CW = 7             # windows per processing chunk
NCHUNK = W // CW   # 14
NODES_PC = W * P   # 12544 dst nodes per core
NPAD = NCORES * NODES_PC  # 100352
TIME_WINDOW = 100

NBANKS = 4         # int16 gather-index banks over x16 rows
BANK = 25089       # rows per bank (<= 32768), NBANKS*BANK >= N
XROWS = NBANKS * BANK

f32 = mybir.dt.float32
f16 = mybir.dt.float16
i32 = mybir.dt.int32
i16 = mybir.dt.int16
OP = mybir.AluOpType


def build_program(B: int):
    """B = blocks per (window, bank); K = NBANKS*B blocks per window."""
    K = NBANKS * B
    C = W * K                    # metadata columns per core
    CBLK = CW * B                # blocks per (chunk, bank)
    NIDX = CBLK * P              # indices per gather call
    ICOLS = NIDX // 16           # idx columns per gather call
    RPC = CW * P                 # rows per chunk
    nc = bacc.Bacc(
        "TRN2", target_bir_lowering=False, debug=False, num_devices=NCORES,
        num_swdge_queues=4,
    )

    # x16 rows are 256 fp16 (512B): 128 features, a 1.0 ones column feeding
    # the count accumulation, then zero padding (dma_gather elem_size must
    # be a multiple of 256B).
    x16 = nc.dram_tensor("x16", [XROWS, 2 * D], f16, kind="ExternalInput")
    # xs/out use the host-permuted row order (chunk, partition, window):
    # row = chunk*CW*P + p*CW + wl, so chunk streams are fully contiguous.
    xs = nc.dram_tensor("xs", [NODES_PC, D], f32, kind="ExternalInput")
    idx16 = nc.dram_tensor(
        "idx16", [P, NCHUNK * NBANKS * ICOLS], i16, kind="ExternalInput"
    )
    etf = nc.dram_tensor("etf", [P, C], f16, kind="ExternalInput")
    stf = nc.dram_tensor("stf", [P, C], f16, kind="ExternalInput")
    dl3 = nc.dram_tensor("dl3", [P, C], f16, kind="ExternalInput")
    out = nc.dram_tensor("out", [NODES_PC, D], f32, kind="ExternalOutput")

    with tile.TileContext(nc) as tc:
        with (
            tc.tile_pool(name="meta", bufs=1) as meta,
            tc.tile_pool(name="sbuf_s", bufs=2) as sbuf_s,
            tc.tile_pool(name="oc", bufs=2) as oc,
            tc.tile_pool(name="small", bufs=4) as small,
            tc.tile_pool(name="psum", bufs=4, space="PSUM") as psum_tp,
        ):
            # ---------------- phase 0: metadata + mask ----------------
            et_t = meta.tile([P, C], f16)
            st_t = meta.tile([P, C], f16)
            dl3_t = meta.tile([P, C], f16)
            idx_t = meta.tile([P, NCHUNK * NBANKS * ICOLS], i16)
            nc.sync.dma_start(out=et_t[:], in_=etf[:])
            nc.sync.dma_start(out=st_t[:], in_=stf[:])
            nc.sync.dma_start(out=dl3_t[:], in_=dl3[:])
            nc.sync.dma_start(out=idx_t[:], in_=idx16[:])

            # iota ramp 0..127 repeated K times: [P, K*P]
            iota_i = meta.tile([P, K * P], i32)
            nc.gpsimd.iota(iota_i[:], pattern=[[0, K], [1, P]], base=0,
                           channel_multiplier=0)
            iota_f = meta.tile([P, K * P], f16)
            nc.vector.tensor_copy(out=iota_f[:], in_=iota_i[:])

            # mask m = (st - et >= 0) & (st - et < TIME_WINDOW); all values
            # are small integers, exact in fp16.
            d_t = meta.tile([P, C], f16)
            nc.vector.tensor_tensor(out=d_t[:], in0=st_t[:], in1=et_t[:],
                                    op=OP.subtract)
            m1_t = meta.tile([P, C], f16)
            nc.vector.tensor_scalar(out=m1_t[:], in0=d_t[:], scalar1=0.0,
                                    scalar2=None, op0=OP.is_ge)
            m2_t = meta.tile([P, C], f16)
            nc.vector.tensor_scalar(out=m2_t[:], in0=d_t[:],
                                    scalar1=float(TIME_WINDOW),
                                    scalar2=None, op0=OP.is_lt)
            m_t = meta.tile([P, C], f16)
            nc.vector.tensor_tensor(out=m_t[:], in0=m1_t[:], in1=m2_t[:],
                                    op=OP.mult)
            # dl_eff = dl3 - 300*m   (in [0,128) iff mask==1)
            m300_t = meta.tile([P, C], f16)
            nc.vector.tensor_scalar(out=m300_t[:], in0=m_t[:], scalar1=300.0,
                                    scalar2=None, op0=OP.mult)
            dle_t = meta.tile([P, C], f16)
            nc.vector.tensor_tensor(out=dle_t[:], in0=dl3_t[:], in1=m300_t[:],
                                    op=OP.subtract)

            # Persistent triple-buffered gather target.  No zero-init is
            # needed: every slot (padding included) gathers a full valid
            # 512B row, so the matmul never reads unwritten bytes.
            g_bufs = []
            for i in range(3):
                g = meta.tile([P, NBANKS * CBLK * 2 * D], f16, tag=f"gbuf{i}")
                g_bufs.append(g)

            # ---------------- main loop ----------------
            for c in range(NCHUNK):
                g_t = g_bufs[c % 3]
                for j in range(NBANKS):
                    icol0 = (c * NBANKS + j) * ICOLS
                    nc.gpsimd.dma_gather(
                        out_ap=g_t[:]
                        .rearrange("p (k c) -> p k c", c=2 * D)[
                            :, j * CBLK : (j + 1) * CBLK, :
                        ],
                        in_ap=x16[j * BANK :, :],
                        idxs_ap=idx_t[:, icol0 : icol0 + ICOLS],
                        num_idxs=NIDX,
                        num_idxs_reg=NIDX,
                        elem_size=2 * D,
                        single_packet=False,
                        queue_num=j,
                    )

                # batched one-hot build for the whole chunk:
                # S[p, (wl k), m] = (iota[m] == dl_eff[p, w*K+k])
                s_t = sbuf_s.tile([P, CW * K * P], f16, tag="s")
                nc.vector.tensor_tensor(
                    out=s_t[:].rearrange("p (w k m) -> p w k m", k=K, m=P),
                    in0=iota_f[:]
                    .rearrange("p (k m) -> p k m", m=P)
                    .unsqueeze(1)
                    .to_broadcast([P, CW, K, P]),
                    in1=dle_t[:, c * CW * K : (c + 1) * CW * K]
                    .rearrange("p (w k) -> p w k", k=K)
                    .unsqueeze(3)
                    .to_broadcast([P, CW, K, P]),
                    op=OP.is_equal,
                )

                # x rows for the residual (contiguous: host-permuted order)
                x_t = oc.tile([P, CW * D], f32, tag="x")
                nc.sync.dma_start(
                    out=x_t[:],
                    in_=xs[c * RPC : (c + 1) * RPC, :].rearrange(
                        "(p w) d -> p (w d)", p=P
                    ),
                )

                o_t = oc.tile([P, CW * D], f32, tag="o")
                for wl in range(CW):
                    ps = psum_tp.tile([P, D + 1], f32, tag="ps")
                    for k in range(K):
                        j, b = divmod(k, B)
                        gblk = j * CBLK + wl * B + b
                        nc.tensor.matmul(
                            out=ps[:],
                            lhsT=s_t[:, (wl * K + k) * P : (wl * K + k + 1) * P],
                            rhs=g_t[:, gblk * 2 * D : gblk * 2 * D + D + 1],
                            start=(k == 0),
                            stop=(k == K - 1),
                        )

                    cnt_t = small.tile([P, 1], f32, tag="cnt")
                    nc.vector.tensor_scalar(out=cnt_t[:], in0=ps[:, D : D + 1],
                                            scalar1=1.0, scalar2=None,
                                            op0=OP.max)
                    rcp_t = small.tile([P, 1], f32, tag="rcp")
                    nc.vector.reciprocal(out=rcp_t[:], in_=cnt_t[:])

                    osl = o_t[:, wl * D : (wl + 1) * D]
                    # mean = psum * (1/cnt) on ACT
                    nc.scalar.activation(
                        out=osl,
                        in_=ps[:, 0:D],
                        func=mybir.ActivationFunctionType.Copy,
                        scale=rcp_t[:, 0:1],
                    )
                    # out = mean + x on DVE
                    nc.vector.tensor_tensor(
                        out=osl, in0=osl, in1=x_t[:, wl * D : (wl + 1) * D],
                        op=OP.add,
                    )

                # store (contiguous: host-permuted row order)
                nc.sync.dma_start(
                    out=out[c * RPC : (c + 1) * RPC, :].rearrange(
                        "(p w) d -> p (w d)", p=P
                    ),
                    in_=o_t[:],
                )

    nc.compile()
    return nc


_PROGRAM_CACHE: dict[int, object] = {}


def _get_program(B: int):
    if B not in _PROGRAM_CACHE:
        _PROGRAM_CACHE[B] = build_program(B)
    return _PROGRAM_CACHE[B]


def _perm_rows(a, nchunk, cw):
    """[nchunk*CW*P, D] row permutation: (c, wl, p) -> (c, p, wl)."""
    return (
        a.reshape(nchunk, cw, P, -1).transpose(0, 2, 1, 3)
        .reshape(nchunk * cw * P, -1)
    )


def _unperm_rows(a, nchunk, cw):
    return (
        a.reshape(nchunk, P, cw, -1).transpose(0, 2, 1, 3)
        .reshape(nchunk * cw * P, -1)
    )


def _prep_inputs(x, edge_index, edge_time, seed_time):
    """Host-side layout: sort edges by (dst window, src bank) into the
    uniform slot grid; build metadata + wrapped int16 gather-index planes."""
    x = np.asarray(x, dtype=np.float32)
    ei = np.asarray(edge_index)
    et = np.asarray(edge_time).astype(np.int64)
    st = np.asarray(seed_time).astype(np.int64)
    N = x.shape[0]
    E = ei.shape[1]
    assert N <= NPAD and N <= XROWS

    src = ei[0].astype(np.int64)
    dst = ei[1].astype(np.int64)

    win = dst // P                      # global window id
    bank = src // BANK                  # 0..NBANKS-1
    gid = win * NBANKS + bank
    order = np.argsort(gid, kind="stable")
    gs = gid[order]
    binc = np.bincount(gid, minlength=NCORES * W * NBANKS)
    B = max(1, int(math.ceil(binc.max() / P)))
    K = NBANKS * B
    C = W * K

    offs = np.zeros(NCORES * W * NBANKS, dtype=np.int64)
    np.cumsum(binc[:-1], out=offs[1:])
    rank = np.arange(E, dtype=np.int64) - offs[gs]  # rank within (window, bank)
    win_s = gs // NBANKS
    bank_s = gs % NBANKS
    core_s = win_s // W
    wloc = win_s % W
    b = rank >> 7
    p = rank & (P - 1)

    # metadata slot grid: col = wloc*K + bank*B + b
    mcol = wloc * K + bank_s * B + b
    et_a = np.zeros((NCORES, P, C), dtype=np.float16)
    st_a = np.full((NCORES, P, C), -2000.0, dtype=np.float16)
    dl3_a = np.full((NCORES, P, C), 1300.0, dtype=np.float16)
    et_a[core_s, p, mcol] = et[order].astype(np.float16)
    st_a[core_s, p, mcol] = st[dst[order]].astype(np.float16)
    dl3_a[core_s, p, mcol] = (dst[order] % P).astype(np.float16) + 300.0

    # gather-index planes: per (chunk, bank) call, position
    # i = ((wl_in_chunk*B) + b)*128 + p, wrapped to [i%16, i//16],
    # replicated across the 8 16-partition groups.
    CBLK = CW * B
    NIDX = CBLK * P
    ICOLS = NIDX // 16
    chunk = wloc // CW
    wl = wloc % CW
    pos = (wl * B + b) * P + p
    icol = (chunk * NBANKS + bank_s) * ICOLS + pos // 16
    irow = pos % 16
    idx_a = np.zeros((NCORES, 16, NCHUNK * NBANKS * ICOLS), dtype=np.int16)
    idx_a[core_s, irow, icol] = (src[order] - bank_s * BANK).astype(np.int16)
    idx_rep = np.tile(idx_a, (1, 8, 1))

    x_pad = np.zeros((NPAD, D), dtype=np.float32)
    x_pad[:N] = x
    x16 = np.zeros((XROWS, 2 * D), dtype=np.float16)
    x16[:N, :D] = x.astype(np.float16)
    x16[:, D] = 1.0  # ones column -> count accumulation rides the matmul
    x_shards = x_pad.reshape(NCORES, NODES_PC, D)

    in_maps = [
        {
            "x16": x16,
            "xs": np.ascontiguousarray(_perm_rows(x_shards[c], NCHUNK, CW)),
            "idx16": idx_rep[c],
            "etf": et_a[c],
            "stf": st_a[c],
            "dl3": dl3_a[c],
        }
        for c in range(NCORES)
    ]
    return in_maps, B, N


def kernel(x, edge_index, edge_time, seed_time):
    in_maps, B, N = _prep_inputs(x, edge_index, edge_time, seed_time)
    nc = _get_program(B)
    res = run_bass_kernel_spmd(nc, in_maps, core_ids=list(range(NCORES)))
    out = np.concatenate(
        [_unperm_rows(res.results[c]["out"], NCHUNK, CW) for c in range(NCORES)],
        axis=0,
    )
    return np.ascontiguousarray(out[:N]).astype(np.float32)



# revision 7
# speedup vs baseline: 2.5611x; 2.5611x over previous
"""Trainium2 Bass kernel for temporal-window GNN mean aggregation.

    out = x + scatter_mean(x[src] * mask, dst),
    mask = (edge_time <= seed_time[dst]) & (edge_time > seed_time[dst] - 100)

Sharding: destination-node sharding across 8 cores (no collectives), with
dst nodes assigned to 128-wide windows in seed_time-sorted order so each
window spans only ~1-2 distinct seed times.  Host work is layout only:
sort nodes by seed_time, sort edges into per-(window, src-bank) slot
grids restricted to each window's conservative candidate time range
(a superset of any possible masked edge for that window), and ship
per-slot metadata (edge_time, seed_time[dst], dst-local index).  All
reference arithmetic — the exact temporal mask compare, the masked
segment sums / counts (one-hot matmul on the PE array), the divide and
the residual add — happens on device.

Device per core (SPMD, one program):
  phase 0: DVE computes the exact mask m per slot and folds it into the
           one-hot key (no iota match -> column contributes 0).
  quarter loop (4 chunks of 24-26 windows):
    - 4x dma_gather on 4 SWDGE queues (one per 25089-row src bank, int16
      index limit) fetch 512-byte x16 rows = [128 features, 1.0, pad]
      for the chunk's slots, bank-major into scratch
    - 16-18 affine SBUF->SBUF DMA copies re-lay scratch into the
      window-major grid (32 slots per bank per window = 128 slots/window)
    - per window: one [128x128]@[128x129] matmul accumulates
      PSUM[dst, 0:129] = S^T @ G (S = one-hot of the masked key); rare
      overflow slots (>32 candidates per window-bank) ride in a tail
      region of quarter 0's gather and add one extra matmul each
    - counts: max(cnt,1) + reciprocal, mean via ACT scale, residual add
      on DVE, store per quarter
"""

import sys

import numpy as np

for _p in ("/opt/trn_rl_repo",):
    if _p not in sys.path:
        sys.path.insert(0, _p)

import concourse.bass as bass
import concourse.mybir as mybir
import concourse.tile as tile
from concourse import bacc
from concourse.bass_utils import run_bass_kernel_spmd

P = 128            # SBUF partitions == dst-window size
D = 128            # feature dim
NCORES = 8
W = 98             # dst windows per core
NODES_PC = W * P   # 12544 dst nodes per core
NPAD = NCORES * NODES_PC  # 100352
TW = 100           # time window

NBANKS = 4         # int16 gather-index banks over x16 rows
BANK = 25089       # rows per bank (<= 32768), NBANKS*BANK >= N
XROWS = NBANKS * BANK
V = 32             # slots per (window, bank); V*NBANKS = 128 = one block
TAILCAP = 64       # overflow slots per (core, bank), in quarter 0's tail
QW = [24, 24, 24, 26]   # windows per quarter-chunk
QBASE = [0, 24, 48, 72]
# slots per (quarter, bank) gather call (mult of 128):
#   q0: 24*32 main + 64 tail + 64 pad = 896 (7 cols); q1/q2: 768 (6 cols)
#   q3: 26*32 = 832 -> 896 with 64 pads (7 cols)
QNIDX = [896, 768, 768, 896]
QCOLS = [n // P for n in QNIDX]
ICOL0 = np.cumsum([0] + [n // 16 for n in QNIDX for _ in range(NBANKS)])
ICOLS_TOT = int(ICOL0[-1])

f32 = mybir.dt.float32
f16 = mybir.dt.float16
i32 = mybir.dt.int32
i16 = mybir.dt.int16
OP = mybir.AluOpType


def build_program(segs: tuple):
    """segs: sorted tuple of (window, bank) overflow segments (union over
    cores); cores without a given overflow see an all-invalid tail key
    there and the extra matmul adds zero."""
    nc = bacc.Bacc(
        "TRN2", target_bir_lowering=False, debug=False, num_devices=NCORES,
        num_swdge_queues=4,
    )

    x16 = nc.dram_tensor("x16", [XROWS, 2 * D], f16, kind="ExternalInput")
    xs = nc.dram_tensor("xs", [NODES_PC, D], f32, kind="ExternalInput")
    idx16 = nc.dram_tensor("idx16", [P, ICOLS_TOT], i16, kind="ExternalInput")
    etg = nc.dram_tensor("etg", [P, W], f16, kind="ExternalInput")
    stg = nc.dram_tensor("stg", [P, W], f16, kind="ExternalInput")
    dlg = nc.dram_tensor("dlg", [P, W], f16, kind="ExternalInput")
    ett = nc.dram_tensor("ett", [P, NBANKS], f16, kind="ExternalInput")
    stt = nc.dram_tensor("stt", [P, NBANKS], f16, kind="ExternalInput")
    dlt = nc.dram_tensor("dlt", [P, NBANKS], f16, kind="ExternalInput")
    wnt = nc.dram_tensor("wnt", [P, NBANKS], f16, kind="ExternalInput")
    out = nc.dram_tensor("out", [NODES_PC, D], f32, kind="ExternalOutput")

    segs_by_w = {}
    for (sw, sj) in segs:
        segs_by_w.setdefault(sw, []).append(sj)

    with tile.TileContext(nc) as tc:
        with (
            tc.tile_pool(name="meta", bufs=1) as meta,
            tc.tile_pool(name="scr0", bufs=1) as scr0p,
            tc.tile_pool(name="scr", bufs=2) as scrp,
            tc.tile_pool(name="grid", bufs=2) as gridp,
            tc.tile_pool(name="sbuf_s", bufs=6) as sp,
            tc.tile_pool(name="oc", bufs=2) as oc,
            tc.tile_pool(name="small", bufs=6) as small,
            tc.tile_pool(name="psum", bufs=6, space="PSUM") as psum_tp,
        ):
            # ---------------- phase 0: metadata + keys ----------------
            et_t = meta.tile([P, W], f16)
            st_t = meta.tile([P, W], f16)
            dl_t = meta.tile([P, W], f16)
            ett_t = meta.tile([P, NBANKS], f16)
            stt_t = meta.tile([P, NBANKS], f16)
            dlt_t = meta.tile([P, NBANKS], f16)
            wnt_t = meta.tile([P, NBANKS], f16)
            idx_t = meta.tile([P, ICOLS_TOT], i16)
            nc.sync.dma_start(out=et_t[:], in_=etg[:])
            nc.sync.dma_start(out=st_t[:], in_=stg[:])
            nc.sync.dma_start(out=dl_t[:], in_=dlg[:])
            nc.sync.dma_start(out=ett_t[:], in_=ett[:])
            nc.sync.dma_start(out=stt_t[:], in_=stt[:])
            nc.sync.dma_start(out=dlt_t[:], in_=dlt[:])
            nc.sync.dma_start(out=wnt_t[:], in_=wnt[:])
            nc.sync.dma_start(out=idx_t[:], in_=idx16[:])

            # iota_f[p, d] = d
            iota_i = meta.tile([P, P], i32)
            nc.gpsimd.iota(iota_i[:], pattern=[[1, P]], base=0,
                           channel_multiplier=0)
            iota_f = meta.tile([P, P], f16)
            nc.vector.tensor_copy(out=iota_f[:], in_=iota_i[:])

            # grid keys: key = dl - 300*m, m = (st-et in [0, TW))
            # (dl stores dst_local + 300; pads store 1300)
            d_g = meta.tile([P, W], f16)
            nc.vector.tensor_tensor(out=d_g[:], in0=st_t[:], in1=et_t[:],
                                    op=OP.subtract)
            m1 = meta.tile([P, W], f16)
            nc.vector.tensor_scalar(out=m1[:], in0=d_g[:], scalar1=0.0,
                                    scalar2=None, op0=OP.is_ge)
            m2 = meta.tile([P, W], f16)
            nc.vector.tensor_scalar(out=m2[:], in0=d_g[:], scalar1=float(TW),
                                    scalar2=None, op0=OP.is_lt)
            m_g = meta.tile([P, W], f16)
            nc.vector.tensor_tensor(out=m_g[:], in0=m1[:], in1=m2[:],
                                    op=OP.mult)
            m300 = meta.tile([P, W], f16)
            nc.vector.tensor_scalar(out=m300[:], in0=m_g[:], scalar1=300.0,
                                    scalar2=None, op0=OP.mult)
            key_g = meta.tile([P, W], f16)
            nc.vector.tensor_tensor(out=key_g[:], in0=dl_t[:], in1=m300[:],
                                    op=OP.subtract)

            # tail keys per bank: kt4 = dl_tail - 300*m_tail + 400
            # (per segment, subtract 400*(win==w) so only that window's
            #  tail slots land in [0,128))
            d_q = meta.tile([P, NBANKS], f16)
            nc.vector.tensor_tensor(out=d_q[:], in0=stt_t[:], in1=ett_t[:],
                                    op=OP.subtract)
            tm1 = meta.tile([P, NBANKS], f16)
            nc.vector.tensor_scalar(out=tm1[:], in0=d_q[:], scalar1=0.0,
                                    scalar2=None, op0=OP.is_ge)
            tm2 = meta.tile([P, NBANKS], f16)
            nc.vector.tensor_scalar(out=tm2[:], in0=d_q[:], scalar1=float(TW),
                                    scalar2=None, op0=OP.is_lt)
            tm = meta.tile([P, NBANKS], f16)
            nc.vector.tensor_tensor(out=tm[:], in0=tm1[:], in1=tm2[:],
                                    op=OP.mult)
            tm300 = meta.tile([P, NBANKS], f16)
            nc.vector.tensor_scalar(out=tm300[:], in0=tm[:], scalar1=300.0,
                                    scalar2=None, op0=OP.mult)
            kt = meta.tile([P, NBANKS], f16)
            nc.vector.tensor_tensor(out=kt[:], in0=dlt_t[:], in1=tm300[:],
                                    op=OP.subtract)
            kt4 = meta.tile([P, NBANKS], f16)
            nc.vector.tensor_scalar(out=kt4[:], in0=kt[:], scalar1=400.0,
                                    scalar2=None, op0=OP.add)

            # ---------------- main loop over quarters ----------------
            scr0 = None
            for q in range(4):
                ncols = QCOLS[q]
                nw = QW[q]
                if q == 0:
                    scr = [scr0p.tile([P, ncols * 2 * D], f16, tag=f"s0b{j}",
                                      name=f"scr0_{j}")
                           for j in range(NBANKS)]
                    scr0 = scr
                else:
                    scr = [scrp.tile([P, ncols * 2 * D], f16, tag=f"sb{j}",
                                     name=f"scr_q{q}_{j}")
                           for j in range(NBANKS)]
                for j in range(NBANKS):
                    icol0 = int(ICOL0[q * NBANKS + j])
                    icn = QNIDX[q] // 16
                    nc.gpsimd.dma_gather(
                        out_ap=scr[j][:]
                        .rearrange("p (k c) -> p k c", c=2 * D),
                        in_ap=x16[j * BANK:, :],
                        idxs_ap=idx_t[:, icol0: icol0 + icn],
                        num_idxs=QNIDX[q],
                        num_idxs_reg=QNIDX[q],
                        elem_size=2 * D,
                        single_packet=False,
                        queue_num=j,
                    )

                # re-lay scratch (bank-major) -> grid (window-major):
                # slot n = w_local*V + r at scratch[(n%128), n//128];
                # V=32 => partition group (w_local%4)*32+r, col w_local//4.
                g_t = gridp.tile([P, nw * 2 * D], f16, tag="g")
                nfull = (nw // 4) * 4
                gv = (g_t[:, 0: nfull * 2 * D]
                      .rearrange("p (w2 g c) -> p w2 g c", g=4, c=2 * D))
                for j in range(NBANKS):
                    sv = scr[j][:].rearrange("p (k c) -> p k c", c=2 * D)
                    for qq in range(4):
                        nc.sync.dma_start(
                            out=gv[32 * j: 32 * j + 32, :, qq, :],
                            in_=sv[32 * qq: 32 * qq + 32, 0: nfull // 4, :],
                        )
                    # leftover windows (nw % 4) live in scratch col nfull//4
                    for wl in range(nfull, nw):
                        qq = wl % 4
                        nc.sync.dma_start(
                            out=(g_t[:]
                                 .rearrange("p (w c) -> p w c", c=2 * D)
                                 [32 * j: 32 * j + 32, wl: wl + 1, :]),
                            in_=sv[32 * qq: 32 * qq + 32,
                                   nfull // 4: nfull // 4 + 1, :],
                        )

                # residual rows for this quarter
                x_t = oc.tile([P, nw * D], f32, tag="x")
                nc.sync.dma_start(
                    out=x_t[:].rearrange("p (w f) -> p w f", f=D),
                    in_=xs[QBASE[q] * P: (QBASE[q] + nw) * P, :].rearrange(
                        "(w d) f -> d w f", d=P
                    ),
                )
                o_t = oc.tile([P, nw * D], f32, tag="o")

                for wl in range(nw):
                    w = QBASE[q] + wl
                    # one-hot S for this window
                    s_t = sp.tile([P, P], f16, tag="s")
                    nc.vector.tensor_tensor(
                        out=s_t[:],
                        in0=iota_f[:],
                        in1=key_g[:, w: w + 1].to_broadcast([P, P]),
                        op=OP.is_equal,
                    )
                    tail_js = segs_by_w.get(w, [])
                    ps = psum_tp.tile([P, D + 1], f32, tag="ps")
                    nc.tensor.matmul(
                        out=ps[:],
                        lhsT=s_t[:],
                        rhs=g_t[:, wl * 2 * D: wl * 2 * D + D + 1],
                        start=True,
                        stop=(len(tail_js) == 0),
                    )
                    for si, sj in enumerate(tail_js):
                        # select this window's tail slots in bank sj
                        v_t = small.tile([P, 1], f16, tag="v")
                        nc.vector.tensor_scalar(
                            out=v_t[:], in0=wnt_t[:, sj: sj + 1],
                            scalar1=float(w), scalar2=400.0,
                            op0=OP.is_equal, op1=OP.mult,
                        )
                        kseg = small.tile([P, 1], f16, tag="k")
                        nc.vector.tensor_tensor(
                            out=kseg[:], in0=kt4[:, sj: sj + 1], in1=v_t[:],
                            op=OP.subtract,
                        )
                        st_s = sp.tile([P, P], f16, tag="st")
                        nc.vector.tensor_tensor(
                            out=st_s[:],
                            in0=iota_f[:],
                            in1=kseg[:].to_broadcast([P, P]),
                            op=OP.is_equal,
                        )
                        nc.tensor.matmul(
                            out=ps[:],
                            lhsT=st_s[:],
                            rhs=scr0[sj][:, 6 * 2 * D: 6 * 2 * D + D + 1],
                            start=False,
                            stop=(si == len(tail_js) - 1),
                        )

                    cnt_t = small.tile([P, 1], f32, tag="cnt")
                    nc.vector.tensor_scalar(out=cnt_t[:], in0=ps[:, D: D + 1],
                                            scalar1=1.0, scalar2=None,
                                            op0=OP.max)
                    rcp_t = small.tile([P, 1], f32, tag="rcp")
                    nc.vector.reciprocal(out=rcp_t[:], in_=cnt_t[:])

                    osl = o_t[:, wl * D: (wl + 1) * D]
                    nc.scalar.activation(
                        out=osl,
                        in_=ps[:, 0:D],
                        func=mybir.ActivationFunctionType.Copy,
                        scale=rcp_t[:, 0:1],
                    )
                    nc.vector.tensor_tensor(
                        out=osl, in0=osl, in1=x_t[:, wl * D: (wl + 1) * D],
                        op=OP.add,
                    )

                nc.sync.dma_start(
                    out=out[QBASE[q] * P: (QBASE[q] + nw) * P, :].rearrange(
                        "(w d) f -> d w f", d=P
                    ),
                    in_=o_t[:].rearrange("p (w f) -> p w f", f=D),
                )

    nc.compile()
    return nc


_PROGRAM_CACHE: dict[tuple, object] = {}


def _get_program(segs: tuple):
    if segs not in _PROGRAM_CACHE:
        _PROGRAM_CACHE[segs] = build_program(segs)
    return _PROGRAM_CACHE[segs]


def _prep_inputs(x, edge_index, edge_time, seed_time):
    """Host-side layout: st-sorted windows, conservative candidate slots,
    uniform V-grid + overflow tails, wrapped int16 gather-index planes."""
    x = np.asarray(x, dtype=np.float32)
    ei = np.asarray(edge_index)
    et = np.asarray(edge_time).astype(np.int64)
    st = np.asarray(seed_time).astype(np.int64)
    N = x.shape[0]
    assert N <= NPAD and N <= XROWS

    src = ei[0].astype(np.int64)
    dst = ei[1].astype(np.int64)

    order = np.argsort(st, kind="stable")
    newid = np.empty(N, np.int64)
    newid[order] = np.arange(N)

    st_pad = np.full(NPAD, -10**6, np.int64)
    st_pad[:N] = st[order]
    wins = st_pad.reshape(-1, P)
    has = (wins > -10**5).any(1)
    st_lo = np.where(has, np.where(wins > -10**5, wins, 10**9).min(1), 0)
    st_hi = np.where(has, np.where(wins > -10**5, wins, -10**9).max(1), -10**6)

    dst_new = newid[dst]
    g_e = dst_new >> 7
    cand = (et > st_lo[g_e] - TW) & (et <= st_hi[g_e])

    csrc = src[cand]
    cet = et[cand]
    cst = st[dst[cand]]
    cg = g_e[cand]
    cdl = dst_new[cand] % P
    cbank = csrc // BANK

    key2 = cg * NBANKS + cbank
    o2 = np.argsort(key2, kind="stable")
    binc = np.bincount(key2, minlength=NCORES * W * NBANKS)
    offs = np.zeros(NCORES * W * NBANKS, np.int64)
    np.cumsum(binc[:-1], out=offs[1:])
    rank = np.empty(len(o2), np.int64)
    rank[o2] = np.arange(len(o2)) - offs[key2[o2]]

    core_e = cg // W
    w_e = cg % W
    is_main = rank < V

    # grid metadata [NCORES, P, W]
    et_g = np.zeros((NCORES, P, W), np.float16)
    st_g = np.full((NCORES, P, W), -2000.0, np.float16)
    dl_g = np.full((NCORES, P, W), 1300.0, np.float16)
    mc, mp, mw = (core_e[is_main], cbank[is_main] * V + rank[is_main],
                  w_e[is_main])
    et_g[mc, mp, mw] = cet[is_main].astype(np.float16)
    st_g[mc, mp, mw] = cst[is_main].astype(np.float16)
    dl_g[mc, mp, mw] = cdl[is_main].astype(np.float16) + 300.0

    # overflow tails [NCORES, P, NBANKS] (partitions 0..TAILCAP-1 used)
    et_a = np.zeros((NCORES, P, NBANKS), np.float16)
    st_a = np.full((NCORES, P, NBANKS), -2000.0, np.float16)
    dl_a = np.full((NCORES, P, NBANKS), 1300.0, np.float16)
    wn_a = np.full((NCORES, P, NBANKS), -1.0, np.float16)

    okey = (core_e * NBANKS + cbank)[~is_main]
    oo = np.argsort(okey, kind="stable")
    obinc = np.bincount(okey, minlength=NCORES * NBANKS)
    assert obinc.max() <= TAILCAP, f"tail overflow: {obinc.max()}"
    ooffs = np.zeros(NCORES * NBANKS, np.int64)
    np.cumsum(obinc[:-1], out=ooffs[1:])
    t_pos = np.empty(len(oo), np.int64)
    t_pos[oo] = np.arange(len(oo)) - ooffs[okey[oo]]
    tc, tj = core_e[~is_main], cbank[~is_main]
    et_a[tc, t_pos, tj] = cet[~is_main].astype(np.float16)
    st_a[tc, t_pos, tj] = cst[~is_main].astype(np.float16)
    dl_a[tc, t_pos, tj] = cdl[~is_main].astype(np.float16) + 300.0
    wn_a[tc, t_pos, tj] = w_e[~is_main].astype(np.float16)

    segs = tuple(sorted(set(zip(w_e[~is_main].tolist(),
                                cbank[~is_main].tolist()))))

    # gather index planes, wrapped [16, n/16], replicated to 128 partitions
    idx_a = np.zeros((NCORES, 16, ICOLS_TOT), np.int16)
    # main slots
    q_of_w = np.zeros(W, np.int64)
    for qi in range(4):
        q_of_w[QBASE[qi]: QBASE[qi] + QW[qi]] = qi
    mq = q_of_w[mw]
    mpos = (mw - np.array(QBASE)[mq]) * V + (mp % V)
    micol = ICOL0[mq * NBANKS + cbank[is_main]] + mpos // 16
    idx_a[mc, mpos % 16, micol] = (csrc[is_main]
                                   - cbank[is_main] * BANK).astype(np.int16)
    # tail slots: quarter 0, positions 768 + t
    tpos = 768 + t_pos
    ticol = ICOL0[0 * NBANKS + tj] + tpos // 16
    idx_a[tc, tpos % 16, ticol] = (csrc[~is_main]
                                   - tj * BANK).astype(np.int16)
    idx_rep = np.tile(idx_a, (1, 8, 1))

    x_pad = np.zeros((NPAD, D), np.float32)
    x_pad[:N] = x[order]
    x16 = np.zeros((XROWS, 2 * D), np.float16)
    x16[:N, :D] = x.astype(np.float16)
    x16[:, D] = 1.0
    x_shards = x_pad.reshape(NCORES, NODES_PC, D)

    in_maps = [
        {
            "x16": x16,
            "xs": np.ascontiguousarray(x_shards[c]),
            "idx16": idx_rep[c],
            "etg": et_g[c], "stg": st_g[c], "dlg": dl_g[c],
            "ett": et_a[c], "stt": st_a[c], "dlt": dl_a[c], "wnt": wn_a[c],
        }
        for c in range(NCORES)
    ]
    return in_maps, segs, order, N


def _postprocess(results, order, N):
    res = np.concatenate([results[c]["out"] for c in range(NCORES)], axis=0)
    out = np.empty((N, D), np.float32)
    out[order] = res[:N]
    return out


def kernel(x, edge_index, edge_time, seed_time):
    in_maps, segs, order, N = _prep_inputs(x, edge_index, edge_time,
                                           seed_time)
    nc = _get_program(segs)
    res = run_bass_kernel_spmd(nc, in_maps, core_ids=list(range(NCORES)))
    return _postprocess(res.results, order, N)
